# revision 29
# baseline (speedup 1.0000x reference)
"""Trainium2 Bass kernel for SAGAN-style self-attention.

Reference computation (per sample, B=8 samples over 8 cores):
    xf = x.reshape(N=4096, C=64)
    f = xf @ Wf + bf            # [N, 8]
    g = xf @ Wg + bg            # [N, 8]
    h = xf @ Wh + bh            # [N, 64]
    s = g @ f.T                 # [N, N]
    beta = softmax(s, axis=-1)
    out = gamma * (beta @ h) + xf

Device-side layout (per core, sample i):
  - st = s.T computed as [m(part), n(free)] tiles; softmax's sum over m
    rides the beta@h matmul as a 65th h-column holding 1/gamma (host-
    prepared), so the o accumulator's row 64 is Z/gamma and a single
    reciprocal gives the combined softmax+gamma scale.
  - exp(st) is split across TWO engines: ScalarE runs exact ACT exp
    into bf16 tiles; VectorE runs a one-instruction Schraudolph exp
    (t = st*A + MAGIC in fp32; the magic-add leaves round(A*st+B) --
    the bf16 bit pattern of e^st -- in the low half-word, consumed by
    the o-matmul through a bitcast stride-2 view).  Max elementwise
    error ~3%; softmax normalization cancels it to ~2e-3 end-to-end.
  - o accumulates over ALL 32 m-chunks of an S-block in one PSUM bank
    (banks 6/7 alternate by S-block parity); the 3-stage epilogue
    copies the bank to SBUF, PE-transposes 128-chunks back into the
    freed bank, then scales by gamma/Z (per-partition reciprocal) and
    adds the residual, staged so no engine FIFO sits on a long wait.
  - PSUM banks 0-5 hold two rotating 3-bank st spans; h production and
    f/g projection borrow banks 6/7 during setup, interleaved with the
    early st spans so phase 1 stays dense on every engine.
"""

import numpy as np

N = 4096
C = 64
D = 8
NCHUNK = 32  # m-chunks of 128
SBLK = 512  # n-block width
NS = N // SBLK  # 8 S-blocks
NCORES = 8

A_CONST = 128.0 / float(np.log(2.0))        # 184.6635
BPRIME = 16256.0 - 6.0  # bf16 exponent bias + sawtooth centering

# spans of m-chunks per S-block, each tagged with its exp engine
# (False = ScalarE exact ACT exp, True = VectorE Schraudolph).  s=0 is
# all-ScalarE (the DVE is busy with f/g copies during setup) and ramps
# 1,2 so the first exps need no f/g partition replicas; later S-blocks
# strictly alternate 3-chunk ScalarE spans with 2-chunk VectorE spans
# so the two engines' exps overlap in the ring.
SPANS = []  # (s, chunk_start, width, is_dve)
for _s in range(NS):
    if _s == 0:
        sizes = [(1, False), (2, False)] + [(3, False)] * 9 + [(2, False)]
    else:
        sizes = []
        for _i in range(6):
            sizes += [(3, False), (2, True)]
        sizes += [(2, False)]
    _c = 0
    for _w, _d in sizes:
        SPANS.append((_s, _c, _w, _d))
        _c += _w
    assert _c == NCHUNK

DVE_SPANS = frozenset(k for k, sp in enumerate(SPANS) if sp[3])

_cache = {}


def _build_nc():
    import concourse.bacc as bacc
    import concourse.tile as tile
    from concourse import mybir

    f32 = mybir.dt.float32
    bf16 = mybir.dt.bfloat16
    EXP = mybir.ActivationFunctionType.Exp
    MULT = mybir.AluOpType.mult
    ADD = mybir.AluOpType.add

    nc = bacc.Bacc("TRN2", target_bir_lowering=False, debug=False)

    id_ext = nc.declare_dram_parameter("ident", [C + 1, C + 1], f32, isOutput=False)
    xr_ext = nc.declare_dram_parameter("xr", [128, NCHUNK, C], f32, isOutput=False)
    xTb_ext = nc.declare_dram_parameter("xTb", [C + 1, N], bf16, isOutput=False)
    whb_ext = nc.declare_dram_parameter("whb", [C + 1, C + 1], bf16, isOutput=False)
    wf_ext = nc.declare_dram_parameter("wf", [C + 1, D], bf16, isOutput=False)
    wg_ext = nc.declare_dram_parameter("wg", [C + 1, D], bf16, isOutput=False)
    out_ext = nc.declare_dram_parameter("out", [N, C], f32, isOutput=True)

    HC = C + 1  # h columns incl. the 1/gamma column

    with tile.TileContext(nc) as tc:
        with (
            tc.tile_pool(name="singles", bufs=1) as singles,
            tc.tile_pool(name="exp_sb", bufs=14) as exp_pool,
            tc.tile_pool(name="dexp_sb", bufs=7) as dexp_pool,
            tc.tile_pool(name="otb_sb", bufs=2) as otb_pool,
            tc.tile_pool(name="tr_sb", bufs=2) as tr_pool,
            tc.tile_pool(name="small", bufs=8) as small,
            tc.tile_pool(name="outsb", bufs=4) as out_pool,
        ):
            # ---- persistent SBUF tensors ----
            x_sb = singles.tile([128, NCHUNK, C], f32)
            wf_sb = singles.tile([C + 1, D], bf16)
            wg_sb = singles.tile([C + 1, D], bf16)
            xTb_sb = singles.tile([C + 1, N], bf16)
            whb_sb = singles.tile([C + 1, HC], bf16)
            fT_sb = singles.tile([128, N], bf16)
            gT_sb = singles.tile([128, N], bf16)
            h_sb = singles.tile([128, NCHUNK, 128], bf16)
            id_sb = singles.tile([C + 1, C + 1], f32)
            dummy = singles.tile([128, 1], f32)
            zt = singles.tile([128, 128], bf16)
            zt2 = singles.tile([128, SBLK], bf16)

            # warm the ACT exp table while input DMAs run
            nc.vector.memset(dummy, 0.0)
            # zero-pad h columns 65-127 so o-matmul weights are 128 wide
            # (enables FWL -> LDWEIGHTS is hidden behind the matmul stream)
            nc.vector.memset(h_sb[:, :, HC:], 0.0)
            nc.vector.memset(zt, 0.0)
            nc.vector.memset(zt2, 0.0)
            nc.scalar.activation(dummy, dummy, EXP)

            # small weights on the gpsimd queue, bulk xT chunked on sync
            nc.gpsimd.dma_start(out=id_sb, in_=id_ext[:])
            nc.gpsimd.dma_start(out=wf_sb, in_=wf_ext[:])
            nc.gpsimd.dma_start(out=wg_sb, in_=wg_ext[:])
            nc.gpsimd.dma_start(out=whb_sb, in_=whb_ext[:])
            for blk in range(NS):
                nc.sync.dma_start(
                    out=xTb_sb[:, blk * SBLK : (blk + 1) * SBLK],
                    in_=xTb_ext[:, blk * SBLK : (blk + 1) * SBLK],
                )
            # residual input, needed from the first epilogue (~20us in)
            nc.gpsimd.dma_start(
                out=x_sb[:, 0 : NCHUNK // 2, :], in_=xr_ext[:, 0 : NCHUNK // 2, :]
            )
            nc.gpsimd.dma_start(
                out=x_sb[:, NCHUNK // 2 :, :], in_=xr_ext[:, NCHUNK // 2 :, :]
            )

            st_psum_cm = tc.tile_pool(name="st_psum", bufs=1, space="PSUM")
            st_psum = st_psum_cm.__enter__()
            # one tensor spanning all 8 PSUM banks; Tile tracks dependencies
            # at bank granularity.  banks 0-5: st spans; 6-7: o accumulators
            # (and, during setup, f/g/h production scratch)
            big = st_psum.tile([128, 8 * SBLK], f32)

            # ~4us of dummy matmuls while input DMAs run: pins the PE's HAM
            # clock-gate at 8/8 (2.4 GHz) before real work, making runtime
            # insensitive to the free-running HAM window phase.
            for _wu in range(12):
                nc.tensor.matmul(
                    big[:, 0:SBLK], lhsT=zt, rhs=zt2, start=True, stop=True
                )


            n_iter = len(SPANS)
            exp_tiles = [None] * n_iter

            first_k_of_s = {}
            for _k, (_s, _c0, _w, _d) in enumerate(SPANS):
                first_k_of_s.setdefault(_s, _k)

            def emit_st(k):
                s, c0, w, _ = SPANS[k]
                base = (k % 2) * 3 * SBLK
                for j in range(w):
                    mc = c0 + j
                    nc.tensor.matmul(
                        big[:, base + j * SBLK : base + (j + 1) * SBLK],
                        lhsT=fT_sb[32 * j : 32 * j + D, mc * 128 : (mc + 1) * 128],
                        rhs=gT_sb[32 * j : 32 * j + D, s * SBLK : (s + 1) * SBLK],
                        start=True,
                        stop=True,
                        tile_position=(32 * j, 0),
                    )
                src = big[:, base : base + w * SBLK]
                if k in DVE_SPANS:
                    expt = dexp_pool.tile([128, 2 * SBLK], mybir.dt.uint16,
                                          tag="dexp")
                    # uint16 writeback rounds-to-nearest: the stored integer
                    # IS the bf16 bit pattern of e^st (Schraudolph)
                    nc.vector.tensor_scalar(
                        out=expt[:, 0 : w * SBLK], in0=src,
                        scalar1=A_CONST, scalar2=BPRIME, op0=MULT, op1=ADD,
                    )
                else:
                    expt = exp_pool.tile([128, 3 * SBLK], bf16, tag="exp")
                    nc.scalar.activation(expt[:, 0 : w * SBLK], src, EXP)
                exp_tiles[k] = expt

            def emit_o(k):
                s, c0, w, _ = SPANS[k]
                expt = exp_tiles[k]
                bank = 6 + (s % 2)
                acc = big[:, bank * SBLK : (bank + 1) * SBLK]
                opening = k == first_k_of_s[s]
                closing = c0 + w == NCHUNK
                if k in DVE_SPANS:
                    bview = expt.bitcast(mybir.dt.bfloat16)
                    rhs_of = lambda j: bview[:, j * SBLK : (j + 1) * SBLK]
                else:
                    rhs_of = lambda j: expt[:, j * SBLK : (j + 1) * SBLK]
                for j in range(w):
                    mc = c0 + j
                    nc.tensor.matmul(
                        acc,
                        lhsT=h_sb[:, mc, :],
                        rhs=rhs_of(j),
                        start=(opening and j == 0),
                        stop=(closing and j == w - 1),
                    )

            def emit_h(t0):
                # four h tiles per call: matmuls into four adjacent sub-slots
                # of one bank (4*65=260 <= 512, no bank crossing), then one
                # strided cast moves all four.  groups alternate banks 6/7
                # and ping-pong between two offsets within each bank.
                g = t0 // 4
                base = (6 + g % 2) * SBLK
                for i, t in enumerate(range(t0, t0 + 4)):
                    hps = big[:, base + i * HC : base + (i + 1) * HC]
                    nc.tensor.matmul(
                        hps,
                        lhsT=xTb_sb[:, t * 128 : (t + 1) * 128],
                        rhs=whb_sb[:],
                        start=True,
                        stop=True,
                    )
                blk4 = big[:, base : base + 4 * HC]
                nc.vector.tensor_copy(
                    h_sb[:, t0 : t0 + 4, 0:HC],
                    blk4.rearrange("p (b x) -> p b x", b=4),
                )

            # epilogue in three stages so neither the DVE FIFO nor the PE
            # ever sits on a long wait: a1 (close+1) copies the acc bank to
            # bf16; a2 (close+2) PE-transposes the 128-chunks back into the
            # just-freed acc bank; b (close+4) copies out, scales by gamma/Z
            # and adds the residual.
            otb_tiles = {}

            def emit_epi_a1(s):
                bank = 6 + (s % 2)
                accv = big[:, bank * SBLK : (bank + 1) * SBLK]
                otb = otb_pool.tile([C + 1, SBLK], f32, tag="otb")
                otb_tiles[s] = otb
                nc.vector.tensor_copy(otb, accv[0 : C + 1, :])

            def emit_epi_a2(s):
                bank = 6 + (s % 2)
                otb = otb_tiles[s]
                for j in range(4):
                    nc.tensor.transpose(
                        big[:, bank * SBLK + j * HC : bank * SBLK + (j + 1) * HC],
                        in_=otb[:, j * 128 : (j + 1) * 128],
                        identity=id_sb[:],
                    )

            def emit_epi_b(s):
                bank = 6 + (s % 2)
                otb_tiles.pop(s)
                tr = tr_pool.tile([128, 4, HC], f32, tag="tr")
                nc.vector.tensor_copy(
                    tr,
                    big[:, bank * SBLK : bank * SBLK + 4 * HC].rearrange(
                        "p (b x) -> p b x", b=4),
                )
                rz4 = small.tile([128, 4, 1], f32, tag="rz")
                nc.vector.reciprocal(rz4, tr[:, :, C : C + 1])
                ot = out_pool.tile([128, 4, C], f32, tag="ot")
                for j in range(4):
                    nc.vector.scalar_tensor_tensor(
                        ot[:, j, :], tr[:, j, 0:C], rz4[:, j],
                        x_sb[:, s * 4 + j, :],
                        MULT, ADD,
                    )
                row = s * 512
                dview = out_ext[row : row + 512, :].rearrange(
                    "(b p) c -> p b c", p=128)
                nc.gpsimd.dma_start(out=dview, in_=ot)

            # ---- f^T and g^T (bias via the ones row of xT_aug) at
            #      partitions 0-7, replicated to 32/64 per chunk via
            #      col-tiled projection matmuls; borrows banks 6 (f) and
            #      7 (g) as PSUM scratch.  st spans are emitted as soon
            #      as their f/g inputs exist ----
            next_st = [0]

            def st_ready(k, b):
                if k >= n_iter:
                    return False
                s, c0, w, _ = SPANS[k]
                return (c0 + w - 1) // 4 <= b and s <= b

            def emit_st_upto(b, limit):
                while next_st[0] <= limit and st_ready(next_st[0], b):
                    emit_st(next_st[0])
                    next_st[0] += 1

            for blk in range(NS):
                for bank, src_w, dst in (
                    (6, wf_sb, fT_sb),
                    (7, wg_sb, gT_sb),
                ):
                    ps = big[:, bank * SBLK : (bank + 1) * SBLK]
                    for j in range(3):
                        nc.tensor.matmul(
                            ps[32 * j : 32 * j + D, :],
                            lhsT=src_w[:],
                            rhs=xTb_sb[:, blk * SBLK : (blk + 1) * SBLK],
                            start=True,
                            stop=True,
                            tile_position=(0, 32 * j),
                        )
                    nc.vector.tensor_copy(
                        dst[0 : 64 + D, blk * SBLK : (blk + 1) * SBLK],
                        ps[0 : 64 + D, :],
                    )
                emit_st_upto(blk, n_iter - 1)
                emit_h(4 * blk)

            stages = {"a1": emit_epi_a1, "a2": emit_epi_a2, "b": emit_epi_b}
            epi_q = []  # (at_k, stage, s) in push order
            for k in range(n_iter):
                emit_st_upto(NS - 1, k + 2)
                while epi_q and epi_q[0][0] <= k:
                    _, stg, s_e = epi_q.pop(0)
                    stages[stg](s_e)
                emit_o(k)
                s, c0, w, _ = SPANS[k]
                if c0 + w == NCHUNK:
                    epi_q += [(k + 1, "a1", s), (k + 2, "a2", s), (k + 4, "b", s)]
            for _, stg, s_e in epi_q:
                stages[stg](s_e)

            st_psum_cm.__exit__(None, None, None)

    nc.finalize()
    return nc


def _get_nc():
    if "nc" not in _cache:
        _cache["nc"] = _build_nc()
    return _cache["nc"]


def make_in_maps(x, kernel_f, kernel_g, kernel_h, bias_f, bias_g, bias_h, gamma):
    from ml_dtypes import bfloat16

    x = np.asarray(x, dtype=np.float32)
    wf_aug = np.concatenate(
        [np.asarray(kernel_f, np.float32).reshape(C, D),
         np.asarray(bias_f, np.float32).reshape(1, D)], axis=0)
    wg_aug = np.concatenate(
        [np.asarray(kernel_g, np.float32).reshape(C, D),
         np.asarray(bias_g, np.float32).reshape(1, D)], axis=0)
    wh_aug = np.concatenate(
        [np.asarray(kernel_h, np.float32).reshape(C, C),
         np.asarray(bias_h, np.float32).reshape(1, C)], axis=0)
    # 65th column: zeros with 1/gamma in the bias row -> h col 64 == 1/gamma,
    # so the o-matmul's Z row accumulates Z/gamma.
    gval = np.float32(np.asarray(gamma).reshape(-1)[0])
    gcol = np.zeros((C + 1, 1), np.float32)
    gcol[C, 0] = np.float32(1.0) / gval
    whb65 = np.concatenate([wh_aug, gcol], axis=1)

    in_maps = []
    for i in range(NCORES):
        xf = x[i].reshape(N, C)
        xr = np.ascontiguousarray(xf.reshape(NCHUNK, 128, C).transpose(1, 0, 2))
        xT_aug = np.concatenate(
            [np.ascontiguousarray(xf.T), np.ones((1, N), np.float32)], axis=0)
        in_maps.append({
            "xr": xr, "xTb": xT_aug.astype(bfloat16),
            "wf": wf_aug.astype(bfloat16), "wg": wg_aug.astype(bfloat16),
            "whb": whb65.astype(bfloat16),
            "ident": np.eye(C + 1, dtype=np.float32),
        })
    return in_maps


def kernel(x, kernel_f, kernel_g, kernel_h, bias_f, bias_g, bias_h, gamma):
    from concourse.bass_utils import run_bass_kernel_spmd

    B, H, W, Cin = x.shape
    assert (B, H, W, Cin) == (8, 64, 64, 64)
    nc = _get_nc()
    in_maps = make_in_maps(x, kernel_f, kernel_g, kernel_h,
                           bias_f, bias_g, bias_h, gamma)
    res = run_bass_kernel_spmd(nc, in_maps, core_ids=list(range(NCORES)))
    out = np.stack([res.results[i]["out"] for i in range(NCORES)], axis=0)
    return out.reshape(B, H, W, Cin).astype(np.float32)


# revision 35
# speedup vs baseline: 1.2677x; 1.2677x over previous
"""Trainium2 Bass kernel for SAGAN-style self-attention.

Reference computation (per sample, B=8 samples over 8 cores):
    xf = x.reshape(N=4096, C=64)
    f = xf @ Wf + bf            # [N, 8]
    g = xf @ Wg + bg            # [N, 8]
    h = xf @ Wh + bh            # [N, 64]
    s = g @ f.T                 # [N, N]
    beta = softmax(s, axis=-1)
    out = gamma * (beta @ h) + xf

Device-side layout (per core, sample i):
  - st = s.T computed as [m(part), n(free)] tiles; softmax's sum over m
    rides the beta@h matmul as a 65th h-column holding 1/gamma (host-
    prepared), so the o accumulator's row 64 is Z/gamma and a single
    reciprocal gives the combined softmax+gamma scale.
  - exp(st) is split across TWO engines: ScalarE runs exact ACT exp
    into bf16 tiles; VectorE runs a one-instruction Schraudolph exp
    (tensor_scalar st*A + B with uint16 output -- the round-to-nearest
    fp32->uint16 writeback makes the stored integer exactly the bf16
    bit pattern of e^st, consumed by the o-matmul through a contiguous
    bitcast).  Max elementwise error ~3%; softmax normalization
    cancels it to ~2e-3 end-to-end.
  - o accumulates over ALL 32 m-chunks of an S-block in one PSUM bank
    (banks 6/7 alternate by S-block parity); the 3-stage epilogue
    copies the bank to SBUF, PE-transposes 128-chunks back into the
    freed bank, then scales by gamma/Z (per-partition reciprocal) and
    adds the residual, staged so no engine FIFO sits on a long wait.
  - PSUM banks 0-5 hold two rotating 3-bank st spans; h production and
    f/g projection borrow banks 6/7 during setup, interleaved with the
    early st spans so phase 1 stays dense on every engine.
"""

import numpy as np

N = 4096
C = 64
D = 8
NCHUNK = 32  # m-chunks of 128
SBLK = 512  # n-block width
NS = N // SBLK  # 8 S-blocks
NCORES = 8

A_CONST = 128.0 / float(np.log(2.0))        # 184.6635
BPRIME = 16256.0 - 6.0  # bf16 exponent bias + sawtooth centering

# spans of m-chunks per S-block, each tagged with its exp engine
# (False = ScalarE exact ACT exp, True = VectorE Schraudolph).  s=0 is
# all-ScalarE (the DVE is busy with f/g copies during setup) and ramps
# 1,2 so the first exps need no f/g partition replicas; later S-blocks
# strictly alternate 3-chunk ScalarE spans with 2-chunk VectorE spans
# so the two engines' exps overlap in the ring.
SPANS = []  # (s, chunk_start, width, is_dve)
for _s in range(NS):
    if _s == 0:
        sizes = [(1, False), (2, False)] + [(3, False)] * 9 + [(2, False)]
    else:
        sizes = []
        for _i in range(6):
            sizes += [(3, False), (2, True)]
        sizes += [(2, False)]
    _c = 0
    for _w, _d in sizes:
        SPANS.append((_s, _c, _w, _d))
        _c += _w
    assert _c == NCHUNK

DVE_SPANS = frozenset(k for k, sp in enumerate(SPANS) if sp[3])

_cache = {}


def _build_nc():
    import concourse.bacc as bacc
    import concourse.tile as tile
    from concourse import mybir

    f32 = mybir.dt.float32
    bf16 = mybir.dt.bfloat16
    EXP = mybir.ActivationFunctionType.Exp
    MULT = mybir.AluOpType.mult
    ADD = mybir.AluOpType.add

    nc = bacc.Bacc("TRN2", target_bir_lowering=False, debug=False)

    id_ext = nc.declare_dram_parameter("ident", [C + 1, C + 1], f32, isOutput=False)
    xr_ext = nc.declare_dram_parameter("xr", [128, NCHUNK, C], f32, isOutput=False)
    xTb_ext = nc.declare_dram_parameter("xTb", [C + 1, N], bf16, isOutput=False)
    whb_ext = nc.declare_dram_parameter("whb", [C + 1, C + 1], bf16, isOutput=False)
    wf_ext = nc.declare_dram_parameter("wf", [C + 1, D], bf16, isOutput=False)
    wg_ext = nc.declare_dram_parameter("wg", [C + 1, D], bf16, isOutput=False)
    out_ext = nc.declare_dram_parameter("out", [N, C], f32, isOutput=True)

    HC = C + 1  # h columns incl. the 1/gamma column

    with tile.TileContext(nc) as tc:
        with (
            tc.tile_pool(name="singles", bufs=1) as singles,
            tc.tile_pool(name="exp_sb", bufs=14) as exp_pool,
            tc.tile_pool(name="dexp_sb", bufs=7) as dexp_pool,
            tc.tile_pool(name="otb_sb", bufs=2) as otb_pool,
            tc.tile_pool(name="tr_sb", bufs=2) as tr_pool,
            tc.tile_pool(name="small", bufs=8) as small,
            tc.tile_pool(name="outsb", bufs=4) as out_pool,
        ):
            # ---- persistent SBUF tensors ----
            x_sb = singles.tile([128, NCHUNK, C], f32)
            wf_sb = singles.tile([C + 1, D], bf16)
            wg_sb = singles.tile([C + 1, D], bf16)
            xTb_sb = singles.tile([C + 1, N], bf16)
            whb_sb = singles.tile([C + 1, HC], bf16)
            fT_sb = singles.tile([128, N], bf16)
            gT_sb = singles.tile([128, N], bf16)
            h_sb = singles.tile([128, NCHUNK, 128], bf16)
            id_sb = singles.tile([C + 1, C + 1], f32)
            dummy = singles.tile([128, 1], f32)

            # warm the ACT exp table while input DMAs run
            nc.vector.memset(dummy, 0.0)
            nc.scalar.activation(dummy, dummy, EXP)

            # small weights on the gpsimd queue, bulk xT chunked on sync
            nc.gpsimd.dma_start(out=id_sb, in_=id_ext[:])
            nc.gpsimd.dma_start(out=wf_sb, in_=wf_ext[:])
            nc.gpsimd.dma_start(out=wg_sb, in_=wg_ext[:])
            nc.gpsimd.dma_start(out=whb_sb, in_=whb_ext[:])
            for blk in range(NS):
                nc.sync.dma_start(
                    out=xTb_sb[:, blk * SBLK : (blk + 1) * SBLK],
                    in_=xTb_ext[:, blk * SBLK : (blk + 1) * SBLK],
                )
            # residual input: the first S-block's epilogue needs chunks 0-3
            # at ~25us, so land those first, then stream the rest
            nc.gpsimd.dma_start(out=x_sb[:, 0:4, :], in_=xr_ext[:, 0:4, :])
            nc.gpsimd.dma_start(out=x_sb[:, 4:16, :], in_=xr_ext[:, 4:16, :])
            nc.gpsimd.dma_start(out=x_sb[:, 16:, :], in_=xr_ext[:, 16:, :])

            st_psum_cm = tc.tile_pool(name="st_psum", bufs=1, space="PSUM")
            st_psum = st_psum_cm.__enter__()
            # one tensor spanning all 8 PSUM banks; Tile tracks dependencies
            # at bank granularity.  banks 0-5: st spans; 6-7: o accumulators
            # (and, during setup, f/g/h production scratch)
            big = st_psum.tile([128, 8 * SBLK], f32)


            n_iter = len(SPANS)
            exp_tiles = [None] * n_iter

            first_k_of_s = {}
            for _k, (_s, _c0, _w, _d) in enumerate(SPANS):
                first_k_of_s.setdefault(_s, _k)

            def emit_st(k):
                s, c0, w, _ = SPANS[k]
                base = (k % 2) * 3 * SBLK
                for j in range(w):
                    mc = c0 + j
                    nc.tensor.matmul(
                        big[:, base + j * SBLK : base + (j + 1) * SBLK],
                        lhsT=fT_sb[32 * j : 32 * j + D, mc * 128 : (mc + 1) * 128],
                        rhs=gT_sb[32 * j : 32 * j + D, s * SBLK : (s + 1) * SBLK],
                        start=True,
                        stop=True,
                        tile_position=(32 * j, 0),
                    )
                src = big[:, base : base + w * SBLK]
                if k in DVE_SPANS:
                    expt = dexp_pool.tile([128, 2 * SBLK], mybir.dt.uint16,
                                          tag="dexp")
                    # uint16 writeback rounds-to-nearest: the stored integer
                    # IS the bf16 bit pattern of e^st (Schraudolph)
                    nc.vector.tensor_scalar(
                        out=expt[:, 0 : w * SBLK], in0=src,
                        scalar1=A_CONST, scalar2=BPRIME, op0=MULT, op1=ADD,
                    )
                else:
                    expt = exp_pool.tile([128, 3 * SBLK], bf16, tag="exp")
                    nc.scalar.activation(expt[:, 0 : w * SBLK], src, EXP)
                exp_tiles[k] = expt

            def emit_o(k):
                s, c0, w, _ = SPANS[k]
                expt = exp_tiles[k]
                bank = 6 + (s % 2)
                acc = big[:, bank * SBLK : (bank + 1) * SBLK]
                opening = k == first_k_of_s[s]
                closing = c0 + w == NCHUNK
                if k in DVE_SPANS:
                    bview = expt.bitcast(mybir.dt.bfloat16)
                    rhs_of = lambda j: bview[:, j * SBLK : (j + 1) * SBLK]
                else:
                    rhs_of = lambda j: expt[:, j * SBLK : (j + 1) * SBLK]
                for j in range(w):
                    mc = c0 + j
                    nc.tensor.matmul(
                        acc,
                        lhsT=h_sb[:, mc, :],
                        rhs=rhs_of(j),
                        start=(opening and j == 0),
                        stop=(closing and j == w - 1),
                    )

            def emit_h(t0):
                # four h tiles per call: matmuls into four adjacent sub-slots
                # of one bank (4*65=260 <= 512, no bank crossing), then one
                # strided cast moves all four.  groups alternate banks 6/7
                # and ping-pong between two offsets within each bank.
                g = t0 // 4
                base = (6 + g % 2) * SBLK
                for i, t in enumerate(range(t0, t0 + 4)):
                    hps = big[:, base + i * HC : base + (i + 1) * HC]
                    nc.tensor.matmul(
                        hps,
                        lhsT=xTb_sb[:, t * 128 : (t + 1) * 128],
                        rhs=whb_sb[:],
                        start=True,
                        stop=True,
                    )
                blk4 = big[:, base : base + 4 * HC]
                nc.vector.tensor_copy(
                    h_sb[:, t0 : t0 + 4, 0:HC],
                    blk4.rearrange("p (b x) -> p b x", b=4),
                )

            # epilogue in three stages so neither the DVE FIFO nor the PE
            # ever sits on a long wait: a1 (close+1) copies the acc bank to
            # bf16; a2 (close+2) PE-transposes the 128-chunks back into the
            # just-freed acc bank; b (close+4) copies out, scales by gamma/Z
            # and adds the residual.
            otb_tiles = {}

            def emit_epi_a1(s):
                bank = 6 + (s % 2)
                accv = big[:, bank * SBLK : (bank + 1) * SBLK]
                otb = otb_pool.tile([C + 1, SBLK], f32, tag="otb")
                otb_tiles[s] = otb
                nc.vector.tensor_copy(otb, accv[0 : C + 1, :])

            def emit_epi_a2(s):
                bank = 6 + (s % 2)
                otb = otb_tiles[s]
                for j in range(4):
                    nc.tensor.transpose(
                        big[:, bank * SBLK + j * HC : bank * SBLK + (j + 1) * HC],
                        in_=otb[:, j * 128 : (j + 1) * 128],
                        identity=id_sb[:],
                    )

            def emit_epi_b(s):
                bank = 6 + (s % 2)
                otb_tiles.pop(s)
                tr = tr_pool.tile([128, 4, HC], f32, tag="tr")
                nc.vector.tensor_copy(
                    tr,
                    big[:, bank * SBLK : bank * SBLK + 4 * HC].rearrange(
                        "p (b x) -> p b x", b=4),
                )
                rz4 = small.tile([128, 4, 1], f32, tag="rz")
                nc.vector.reciprocal(rz4, tr[:, :, C : C + 1])
                ot = out_pool.tile([128, 4, C], f32, tag="ot")
                for j in range(4):
                    nc.vector.scalar_tensor_tensor(
                        ot[:, j, :], tr[:, j, 0:C], rz4[:, j],
                        x_sb[:, s * 4 + j, :],
                        MULT, ADD,
                    )
                row = s * 512
                dview = out_ext[row : row + 512, :].rearrange(
                    "(b p) c -> p b c", p=128)
                nc.gpsimd.dma_start(out=dview, in_=ot)

            # ---- f^T and g^T (bias via the ones row of xT_aug) at
            #      partitions 0-7, replicated to 32/64 per chunk via
            #      col-tiled projection matmuls; borrows banks 6 (f) and
            #      7 (g) as PSUM scratch.  st spans are emitted as soon
            #      as their f/g inputs exist ----
            next_st = [0]

            def st_ready(k, b):
                if k >= n_iter:
                    return False
                s, c0, w, _ = SPANS[k]
                return (c0 + w - 1) // 4 <= b and s <= b

            def emit_st_upto(b, limit):
                while next_st[0] <= limit and st_ready(next_st[0], b):
                    emit_st(next_st[0])
                    next_st[0] += 1

            for blk in range(NS):
                for bank, src_w, dst in (
                    (6, wf_sb, fT_sb),
                    (7, wg_sb, gT_sb),
                ):
                    ps = big[:, bank * SBLK : (bank + 1) * SBLK]
                    for j in range(3):
                        nc.tensor.matmul(
                            ps[32 * j : 32 * j + D, :],
                            lhsT=src_w[:],
                            rhs=xTb_sb[:, blk * SBLK : (blk + 1) * SBLK],
                            start=True,
                            stop=True,
                            tile_position=(0, 32 * j),
                        )
                    nc.vector.tensor_copy(
                        dst[0 : 64 + D, blk * SBLK : (blk + 1) * SBLK],
                        ps[0 : 64 + D, :],
                    )
                emit_st_upto(blk, n_iter - 1)
                emit_h(4 * blk)
                if blk == 0:
                    # zero-pad h cols 65-127 (128-wide o-matmul weights ->
                    # FWL hides LDWEIGHTS).  Emitted after block 0 so this
                    # ~2us strided memset doesn't sit in the DVE FIFO ahead
                    # of the f/g copies gating the first st span; it is only
                    # needed before the first o-matmul, deep in phase 2.
                    nc.vector.memset(h_sb[:, :, HC:], 0.0)

            stages = {"a1": emit_epi_a1, "a2": emit_epi_a2, "b": emit_epi_b}
            epi_q = []  # (at_k, stage, s) in push order
            for k in range(n_iter):
                emit_st_upto(NS - 1, k + 2)
                while epi_q and epi_q[0][0] <= k:
                    _, stg, s_e = epi_q.pop(0)
                    stages[stg](s_e)
                emit_o(k)
                s, c0, w, _ = SPANS[k]
                if c0 + w == NCHUNK:
                    epi_q += [(k + 1, "a1", s), (k + 2, "a2", s), (k + 4, "b", s)]
            for _, stg, s_e in epi_q:
                stages[stg](s_e)

            st_psum_cm.__exit__(None, None, None)

    nc.finalize()
    return nc


def _get_nc():
    if "nc" not in _cache:
        _cache["nc"] = _build_nc()
    return _cache["nc"]


def make_in_maps(x, kernel_f, kernel_g, kernel_h, bias_f, bias_g, bias_h, gamma):
    from ml_dtypes import bfloat16

    x = np.asarray(x, dtype=np.float32)
    wf_aug = np.concatenate(
        [np.asarray(kernel_f, np.float32).reshape(C, D),
         np.asarray(bias_f, np.float32).reshape(1, D)], axis=0)
    wg_aug = np.concatenate(
        [np.asarray(kernel_g, np.float32).reshape(C, D),
         np.asarray(bias_g, np.float32).reshape(1, D)], axis=0)
    wh_aug = np.concatenate(
        [np.asarray(kernel_h, np.float32).reshape(C, C),
         np.asarray(bias_h, np.float32).reshape(1, C)], axis=0)
    # 65th column: zeros with 1/gamma in the bias row -> h col 64 == 1/gamma,
    # so the o-matmul's Z row accumulates Z/gamma.
    gval = np.float32(np.asarray(gamma).reshape(-1)[0])
    gcol = np.zeros((C + 1, 1), np.float32)
    gcol[C, 0] = np.float32(1.0) / gval
    whb65 = np.concatenate([wh_aug, gcol], axis=1)

    in_maps = []
    for i in range(NCORES):
        xf = x[i].reshape(N, C)
        xr = np.ascontiguousarray(xf.reshape(NCHUNK, 128, C).transpose(1, 0, 2))
        xT_aug = np.concatenate(
            [np.ascontiguousarray(xf.T), np.ones((1, N), np.float32)], axis=0)
        in_maps.append({
            "xr": xr, "xTb": xT_aug.astype(bfloat16),
            "wf": wf_aug.astype(bfloat16), "wg": wg_aug.astype(bfloat16),
            "whb": whb65.astype(bfloat16),
            "ident": np.eye(C + 1, dtype=np.float32),
        })
    return in_maps


def kernel(x, kernel_f, kernel_g, kernel_h, bias_f, bias_g, bias_h, gamma):
    from concourse.bass_utils import run_bass_kernel_spmd

    B, H, W, Cin = x.shape
    assert (B, H, W, Cin) == (8, 64, 64, 64)
    nc = _get_nc()
    in_maps = make_in_maps(x, kernel_f, kernel_g, kernel_h,
                           bias_f, bias_g, bias_h, gamma)
    res = run_bass_kernel_spmd(nc, in_maps, core_ids=list(range(NCORES)))
    out = np.stack([res.results[i]["out"] for i in range(NCORES)], axis=0)
    return out.reshape(B, H, W, Cin).astype(np.float32)
